# revision 20
# baseline (speedup 1.0000x reference)
"""Trainium2 Bass kernel for a DEQ Transformer-XL layer (relative attention + FFN).

Sharding (8 cores): 2 batch groups x 4-way tensor parallel.
  core c: batch b = c//4, group g = c%4
  - qkv/r projections + attention: heads [4g, 4g+4) (Megatron column split)
  - o_net: partial over this core's 256 channels -> ReduceScatter(token-sharded)
  - LN on this core's 256-token slice -> AllGather -> FFN with d_inner shard
    [1024g, 1024g+1024) -> ReduceScatter -> final LN on token slice.
Rel-shift (Transformer-XL BD term) is realized as a sloped-stride DMA read from
a DRAM scratch of the per-(head,i-chunk) position-score matrix.
"""

from contextlib import ExitStack

import numpy as np
import ml_dtypes

import concourse.bass as bass
import concourse.mybir as mybir
import concourse.tile as tile
from concourse import bacc
from concourse import bass_utils

F32 = mybir.dt.float32
F32R = mybir.dt.float32r
BF16 = mybir.dt.bfloat16
AF = mybir.ActivationFunctionType

B, D, Q, M = 2, 1024, 1024, 1024
K = Q + M                 # 2048
KP = K + 128              # padded key extent (2176) so every i-chunk has a full span
NHL = 4                   # heads per core
DH = 64
CO = NHL * DH             # 256 channels per core
DIL = 1024                # d_inner shard per core
TL = 256                  # token slice per core (within batch group)
SPAN = 1280               # aligned j-span per 128-query chunk
NJC = SPAN // 128         # 10
PREW = 1408               # padded pre-matrix width: 153 left pad + 1000 + 255 right pad
PREOFF = 153
EPS = 1e-5
NEG = -1e30
RG = [[0, 1, 2, 3], [4, 5, 6, 7]]
N_CORES = 8

_CACHE = {}


def _build():
    nc = bacc.Bacc("TRN2", target_bir_lowering=False, debug=False,
                   num_devices=N_CORES)

    def din(name, shape, dt=BF16):
        return nc.dram_tensor(name, shape, dt, kind="ExternalInput").ap()

    cat = din("cat", [D, K])
    wqT = din("wqT", [D, CO])
    wkT = din("wkT", [D, CO])
    wvT = din("wvT", [D, CO])
    wrT = din("wrT", [D, CO])
    u_q = din("u_q", [CO, Q])
    u_k = din("u_k", [CO, K])
    u_vT = din("u_vT", [K, CO])
    pos_w = din("pos_w", [D, 1000])
    ident_b = din("ident_b", [128, 128], BF16)
    ones_b = din("ones_b", [128, 1], BF16)
    onesr = din("onesr", [1, 128], F32)
    rwb = din("rwb", [128, 2], F32)
    rrb = din("rrb", [128, 2], F32)
    woT = din("woT", [CO, D], BF16)
    bo = din("bo", [128, 8], F32)
    zres = din("zres", [D, TL], F32)
    wff1T = din("wff1T", [D, DIL], BF16)
    bff1 = din("bff1", [128, 8], F32)
    wff2T = din("wff2T", [DIL, D], BF16)
    bff2 = din("bff2", [128, 8], F32)

    y_out = nc.dram_tensor("y", [D, TL], F32, kind="ExternalOutput").ap()

    with tile.TileContext(nc) as tc, ExitStack() as stack:
        consts = stack.enter_context(tc.tile_pool(name="consts", bufs=1))
        id_b = consts.tile([128, 128], BF16)
        nc.gpsimd.dma_start(out=id_b, in_=ident_b)
        ones_sb = consts.tile([128, 1], BF16)
        nc.gpsimd.dma_start(out=ones_sb, in_=ones_b)
        onesr_sb = consts.tile([1, 128], F32)
        nc.gpsimd.dma_start(out=onesr_sb, in_=onesr)
        rwb_sb = consts.tile([128, 2], F32)
        nc.gpsimd.dma_start(out=rwb_sb, in_=rwb)
        rrb_sb = consts.tile([128, 2], F32)
        nc.gpsimd.dma_start(out=rrb_sb, in_=rrb)
        bo_sb = consts.tile([128, 8], F32)
        nc.gpsimd.dma_start(out=bo_sb, in_=bo)
        bff1_sb = consts.tile([128, 8], F32)
        nc.gpsimd.dma_start(out=bff1_sb, in_=bff1)
        bff2_sb = consts.tile([128, 8], F32)
        nc.gpsimd.dma_start(out=bff2_sb, in_=bff2)
        eps_sb = consts.tile([1, 1], F32)
        nc.vector.memset(eps_sb, EPS)
        cw_sb = consts.tile([128, 4], F32)

        # persistent activations
        acts = stack.enter_context(tc.tile_pool(name="acts", bufs=1))
        qp = [acts.tile([128, Q], BF16, tag=f"qp{i}", name=f"qp{i}") for i in range(2)]
        qpp = [acts.tile([128, Q], BF16, tag=f"qpp{i}", name=f"qpp{i}") for i in range(2)]
        kk = [acts.tile([128, KP], BF16, tag=f"kk{i}", name=f"kk{i}") for i in range(2)]
        rk = [acts.tile([128, 1000], BF16, tag=f"rk{i}", name=f"rk{i}") for i in range(2)]
        vT = acts.tile([128, KP // 128, CO], BF16)
        av = [acts.tile([128, Q], BF16, tag=f"av{i}", name=f"av{i}") for i in range(2)]
        hsl = acts.tile([128, 8, TL], F32)        # post-attn LN token slice
        hbf = acts.tile([128, 8, TL], BF16)       # bf16 copy for AG1

        dram = stack.enter_context(tc.tile_pool(name="dram", bufs=4, space="DRAM"))
        HD = D // 2
        rs1_in = [dram.tile([4, HD, TL], BF16, tag=f"rs1i{i}", name=f"rs1i{i}") for i in range(2)]
        rs1_out = [dram.tile([HD, TL], BF16, tag=f"rs1o{i}", name=f"rs1o{i}") for i in range(2)]
        ag1_in = [dram.tile([HD, TL], BF16, tag=f"ag1i{i}", name=f"ag1i{i}") for i in range(2)]
        ag1_out = [dram.tile([4, HD, TL], BF16, tag=f"ag1o{i}", name=f"ag1o{i}") for i in range(2)]
        rs2_in = [dram.tile([4, HD, TL], BF16, tag=f"rs2i{i}", name=f"rs2i{i}") for i in range(2)]
        rs2_out = [dram.tile([HD, TL], BF16, tag=f"rs2o{i}", name=f"rs2o{i}") for i in range(2)]
        dscr = dram.tile([32, 128, PREW], BF16, tag="dscr")

        # warmup collective: absorb ncfw first-call overhead under phase 1
        cw_in = dram.tile([128, 4], F32, tag="cwi")
        cw_out = dram.tile([128, 4], F32, tag="cwo")
        nc.vector.memset(cw_sb, 0.0)
        nc.sync.dma_start(out=cw_in, in_=cw_sb)
        nc.gpsimd.collective_compute(
            "AllReduce", mybir.AluOpType.add, replica_groups=RG,
            ins=[cw_in[:]], outs=[cw_out[:]])

        # ---------------- Phase 1: projections + BD pre ----------------
        with tc.tile_pool(name="p1w", bufs=3) as wp, \
             tc.tile_pool(name="p1c", bufs=1) as cp, \
             tc.tile_pool(name="p1ps", bufs=2, space="PSUM") as pp:
            cat_sb = [cp.tile([128, K], BF16, tag=f"cat{dc}", name=f"cat{dc}") for dc in range(8)]
            for dc in range(8):
                nc.gpsimd.dma_start(out=cat_sb[dc], in_=cat[dc * 128:(dc + 1) * 128, :])

            def proj_qk(wT, usrc, toff, tlen, is_q):
                w_sb = [wp.tile([128, CO], BF16, tag=f"w1_{dc}", name="w1t") for dc in range(8)]
                for dc in range(8):
                    weng = nc.sync if dc % 2 == 0 else nc.scalar
                    weng.dma_start(out=w_sb[dc], in_=wT[dc * 128:(dc + 1) * 128, :])
                for oc in range(2):
                    for nb in range(tlen // 512):
                        ps = pp.tile([128, 512], F32, tag="ps1", name="ps1")
                        for dc in range(8):
                            nc.tensor.matmul(
                                ps,
                                lhsT=w_sb[dc][:, oc * 128:(oc + 1) * 128],
                                rhs=cat_sb[dc][:, toff + nb * 512: toff + nb * 512 + 512],
                                start=(dc == 0), stop=False)
                        uu = wp.tile([128, 512], BF16, tag="u1", name="uu")
                        nc.sync.dma_start(
                            out=uu,
                            in_=usrc[oc * 128:(oc + 1) * 128, nb * 512:nb * 512 + 512])
                        nc.tensor.matmul(ps, lhsT=id_b, rhs=uu, start=False, stop=True)
                        sl = (slice(None), slice(nb * 512, nb * 512 + 512))
                        if is_q:
                            nc.vector.tensor_scalar_add(qp[oc][sl], ps, rwb_sb[:, oc:oc + 1])
                            nc.vector.tensor_scalar_add(qpp[oc][sl], ps, rrb_sb[:, oc:oc + 1])
                        else:
                            nc.scalar.copy(kk[oc][sl], ps)

            # q projection first (pre-phase depends on it)
            proj_qk(wqT, u_q, M, Q, True)

            # r_k projection
            wr_sb = [wp.tile([128, CO], BF16, tag=f"w1_{dc}", name="wrt") for dc in range(8)]
            pos_sb = [cp.tile([128, 1000], BF16, tag=f"pos{dc}", name=f"pos{dc}") for dc in range(8)]
            for dc in range(8):
                nc.sync.dma_start(out=wr_sb[dc], in_=wrT[dc * 128:(dc + 1) * 128, :])
                nc.gpsimd.dma_start(out=pos_sb[dc], in_=pos_w[dc * 128:(dc + 1) * 128, :])
            for oc in range(2):
                for nb in range(2):
                    ps = pp.tile([128, 512], F32, tag="ps1", name="ps1")
                    for dc in range(8):
                        nc.tensor.matmul(
                            ps[:, :500],
                            lhsT=wr_sb[dc][:, oc * 128:(oc + 1) * 128],
                            rhs=pos_sb[dc][:, nb * 500:nb * 500 + 500],
                            start=(dc == 0), stop=(dc == 7))
                    nc.scalar.copy(rk[oc][:, nb * 500:nb * 500 + 500], ps[:, :500])

            # BD pre-matrices (overlaps the k/v projections below)
            for h in range(NHL):
                ht, hh = h // 2, (h % 2) * 64
                hsla = slice(hh, hh + 64)
                for ic in range(8):
                    i0 = 128 * ic
                    P = pp.tile([128, 1024], F32, tag="ppre", name="P", bufs=2)
                    for nb in range(2):
                        nc.tensor.matmul(
                            P[:, nb * 512:nb * 512 + 500],
                            lhsT=qpp[ht][hsla, i0:i0 + 128],
                            rhs=rk[ht][hsla, nb * 500:nb * 500 + 500],
                            start=True, stop=True)
                    pre = wp.tile([128, PREW], BF16, tag="pre", name="pre")
                    nc.vector.memset(pre[:, 0:PREOFF], NEG)
                    nc.vector.memset(pre[:, PREOFF + 1000:PREW], NEG)
                    ceng = nc.scalar if (ic % 2 == 0) else nc.vector
                    oeng = nc.vector if (ic % 2 == 0) else nc.scalar
                    ceng.copy(pre[:, PREOFF:PREOFF + 500], P[:, 0:500]) if ceng is nc.scalar \
                        else ceng.tensor_copy(pre[:, PREOFF:PREOFF + 500], P[:, 0:500])
                    oeng.copy(pre[:, PREOFF + 500:PREOFF + 1000], P[:, 512:1012]) if oeng is nc.scalar \
                        else oeng.tensor_copy(pre[:, PREOFF + 500:PREOFF + 1000], P[:, 512:1012])
                    nc.gpsimd.dma_start(out=dscr[h * 8 + ic], in_=pre)

            # k projection
            proj_qk(wkT, u_k, 0, K, False)
            for oc in range(2):
                nc.vector.memset(kk[oc][:, K:KP], 0.0)

            # v projection (activation-stationary, out (t, o))
            wv_sb = [wp.tile([128, CO], BF16, tag=f"w1_{dc}", name="wvt") for dc in range(8)]
            for dc in range(8):
                weng = nc.sync if dc % 2 == 0 else nc.scalar
                weng.dma_start(out=wv_sb[dc], in_=wvT[dc * 128:(dc + 1) * 128, :])
            for tch in range(16):
                ps = pp.tile([128, 256], F32, tag="psv", name="psv")
                for dc in range(8):
                    nc.tensor.matmul(
                        ps,
                        lhsT=cat_sb[dc][:, tch * 128:(tch + 1) * 128],
                        rhs=wv_sb[dc],
                        start=(dc == 0), stop=False)
                uu = wp.tile([128, 256], BF16, tag="uv", name="uuv")
                nc.sync.dma_start(out=uu, in_=u_vT[tch * 128:(tch + 1) * 128, :])
                nc.tensor.matmul(ps, lhsT=id_b, rhs=uu, start=False, stop=True)
                nc.scalar.copy(vT[:, tch, :], ps)
            nc.vector.memset(vT[:, 16, :], 0.0)

        # ---------------- Phase 2: attention (head-paired) ----------------
        NBLK = ((0, 512), (512, 512), (1024, 256))
        with tc.tile_pool(name="p2s", bufs=4) as sp, \
             tc.tile_pool(name="p2ps", bufs=2, space="PSUM") as ppS, \
             tc.tile_pool(name="p2pa", bufs=2, space="PSUM") as ppA:
            pending = None

            def emit_av(ht, ic, pT0, pT1):
                AVp0 = ppA.tile([64, 128], F32, tag="avp0", name="AVp0", bufs=1)
                AVp1 = ppA.tile([64, 128], F32, tag="avp1", name="AVp1", bufs=1)
                for c in range(NJC):
                    nc.tensor.matmul(AVp0,
                                     lhsT=vT[:, ic + c, ht * 128:ht * 128 + 64],
                                     rhs=pT0[:, c, :],
                                     start=(c == 0), stop=(c == NJC - 1))
                    nc.tensor.matmul(AVp1,
                                     lhsT=vT[:, ic + c, ht * 128 + 64:ht * 128 + 128],
                                     rhs=pT1[:, c, :],
                                     start=(c == 0), stop=(c == NJC - 1))
                nc.scalar.copy(av[ht][0:64, 128 * ic:128 * ic + 128], AVp0)
                nc.scalar.copy(av[ht][64:128, 128 * ic:128 * ic + 128], AVp1)

            for ic in range(8):
                i0 = 128 * ic
                for ht in range(2):
                    pTs = []
                    Ss = []
                    for hp in range(2):
                        h = 2 * ht + hp
                        hsla = slice(hp * 64, hp * 64 + 64)
                        idx = h * 8 + ic
                        bd = sp.tile([128, SPAN], BF16, tag=f"bd{hp}", name="bd")
                        diag = bass.AP(tensor=dscr.tensor,
                                       offset=dscr.offset + idx * 128 * PREW + PREOFF,
                                       ap=[[PREW - 1, 128], [1, SPAN]])
                        nc.gpsimd.dma_start(out=bd, in_=diag)
                        S = ppS.tile([128, SPAN], F32, tag=f"S{hp}", name="S", bufs=1)
                        for c0, cw in NBLK:
                            nc.tensor.matmul(
                                S[:, c0:c0 + cw],
                                lhsT=qp[ht][hsla, i0:i0 + 128],
                                rhs=kk[ht][hsla, i0 + c0:i0 + c0 + cw],
                                start=True, stop=True)
                        nc.vector.tensor_add(S, S, bd)
                        Ss.append(S)
                    for hp, S in enumerate(Ss):
                        prob = sp.tile([128, SPAN], BF16, tag=f"prob{hp}", name="prob")
                        rsum = sp.tile([128, 1], F32, tag=f"rsum{hp}", name="rsum")
                        nc.scalar.activation(out=prob, in_=S, func=AF.Exp,
                                             scale=0.125, accum_out=rsum)
                        rinv = sp.tile([128, 1], F32, tag=f"rinv{hp}", name="rinv")
                        nc.vector.reciprocal(rinv, rsum)
                        nc.vector.tensor_scalar_mul(prob, prob, rinv)
                        pT = sp.tile([128, NJC, 128], BF16, tag=f"pT{hp}", name="pT")
                        xeng = nc.sync if hp == 0 else nc.scalar
                        xeng.dma_start_transpose(pT, prob)
                        pTs.append(pT)
                    if pending is not None:
                        emit_av(*pending)
                    pending = (ht, ic, pTs[0], pTs[1])
            emit_av(*pending)

        # ---------------- Phase 3: o_net partial + RS1 ----------------
        with tc.tile_pool(name="p3w", bufs=2) as wp3, \
             tc.tile_pool(name="p3s", bufs=3) as sp3, \
             tc.tile_pool(name="p3ps", bufs=2, space="PSUM") as pp3:
            wo_sb = [wp3.tile([128, D], BF16, tag="wo", name="wo") for _ in range(2)]
            for cc in range(2):
                nc.sync.dma_start(out=wo_sb[cc], in_=woT[cc * 128:(cc + 1) * 128, :])
            for half in range(2):
                for oc4 in range(4):
                    oc = half * 4 + oc4
                    for nb in range(2):
                        ps = pp3.tile([128, 512], F32, tag="ps3")
                        for cc in range(2):
                            nc.tensor.matmul(ps,
                                             lhsT=wo_sb[cc][:, oc * 128:(oc + 1) * 128],
                                             rhs=av[cc][:, nb * 512:nb * 512 + 512],
                                             start=(cc == 0), stop=(cc == 1))
                        ot = sp3.tile([128, 512], BF16, tag="ot")
                        nc.scalar.copy(ot, ps)
                        dst = rs1_in[half][nb * 2:nb * 2 + 2, oc4 * 128:(oc4 + 1) * 128, :] \
                            .rearrange("b o t -> o b t")
                        nc.sync.dma_start(out=dst, in_=ot)
                nc.gpsimd.collective_compute(
                    "ReduceScatter", mybir.AluOpType.add, replica_groups=RG,
                    ins=[rs1_in[half][:]], outs=[rs1_out[half][:]])

        # ---------------- Phase 4: post-attn LN on token slice ----------------
        def layer_norm(pool, psum_pool, x, out_f32, out_bf16):
            """x: (128, 8, TL) f32 tile; writes normalized to out tiles."""
            xb16 = pool.tile([128, 8, TL], BF16, tag="lnxb", name="lnxb")
            nc.vector.tensor_copy(xb16, x)
            sq = pool.tile([128, 8, TL], BF16, tag="lnsq", name="lnsq")
            nc.vector.tensor_mul(sq, xb16, xb16)
            Sp = psum_pool.tile([1, 2 * TL], F32, tag="lnps", name="lnps")
            for dc in range(8):
                nc.tensor.matmul(Sp[:, 0:TL], lhsT=ones_sb, rhs=xb16[:, dc, :],
                                 start=(dc == 0), stop=(dc == 7))
            for dc in range(8):
                nc.tensor.matmul(Sp[:, TL:2 * TL], lhsT=ones_sb, rhs=sq[:, dc, :],
                                 start=(dc == 0), stop=(dc == 7))
            st = pool.tile([1, 2 * TL], F32, tag="lnst", name="lnst")
            # st[0:TL] = -mean ; st[TL:2TL] = rstd
            nc.vector.tensor_scalar_mul(st[:, 0:TL], Sp[:, 0:TL], -1.0 / D)
            m2 = pool.tile([1, TL], F32, tag="lnm2", name="lnm2")
            nc.vector.tensor_scalar_mul(m2, Sp[:, TL:2 * TL], 1.0 / D)
            msq = pool.tile([1, TL], F32, tag="lnmsq", name="lnmsq")
            nc.vector.tensor_mul(msq, st[:, 0:TL], st[:, 0:TL])
            var = pool.tile([1, TL], F32, tag="lnvar", name="lnvar")
            nc.vector.tensor_sub(var, m2, msq)
            sd = pool.tile([1, TL], F32, tag="lnsd", name="lnsd")
            nc.scalar.activation(out=sd, in_=var, func=AF.Sqrt, bias=eps_sb, scale=1.0)
            nc.vector.reciprocal(st[:, TL:2 * TL], sd)
            bcp = psum_pool.tile([128, 2 * TL], F32, tag="lnbc", name="lnbc")
            nc.tensor.matmul(bcp, lhsT=onesr_sb, rhs=st, start=True, stop=True)
            nm = bass.AP(tensor=bcp.tensor, offset=bcp.offset,
                         ap=[bcp.ap[0], [0, 8], [1, TL]])
            rs = bass.AP(tensor=bcp.tensor, offset=bcp.offset + TL,
                         ap=[bcp.ap[0], [0, 8], [1, TL]])
            cen = pool.tile([128, 8, TL], F32, tag="lncen", name="lncen")
            nc.vector.tensor_add(cen, x, nm)
            if out_bf16 is not None:
                nc.vector.tensor_mul(out_bf16, cen, rs)
            if out_f32 is not None:
                nc.vector.tensor_mul(out_f32, cen, rs)

        with tc.tile_pool(name="p4s", bufs=1) as sp4, \
             tc.tile_pool(name="p4ps", bufs=1, space="PSUM") as pp4:
            xb = sp4.tile([128, 8, TL], BF16, tag="xb")
            for half in range(2):
                nc.sync.dma_start(
                    out=xb[:, half * 4:half * 4 + 4, :],
                    in_=rs1_out[half].rearrange("(c p) t -> p c t", p=128))
            xat = sp4.tile([128, 8, TL], F32, tag="xat")
            nc.vector.tensor_copy(xat, xb)
            zs = sp4.tile([128, 8, TL], F32, tag="zs")
            nc.sync.dma_start(out=zs, in_=zres.rearrange("(c p) t -> p c t", p=128))
            nc.vector.tensor_add(xat, xat, zs)
            layer_norm(sp4, pp4, xat, hsl, hbf)
            for half in range(2):
                nc.sync.dma_start(
                    out=ag1_in[half].rearrange("(c p) t -> p c t", p=128),
                    in_=hbf[:, half * 4:half * 4 + 4, :])
                nc.gpsimd.collective_compute(
                    "AllGather", mybir.AluOpType.bypass, replica_groups=RG,
                    ins=[ag1_in[half][:]], outs=[ag1_out[half][:]])

        # ---------------- Phase 5: FFN ----------------
        with tc.tile_pool(name="p5w", bufs=1) as wp5, \
             tc.tile_pool(name="p5s", bufs=2) as sp5, \
             tc.tile_pool(name="p5ps", bufs=3, space="PSUM") as pp5:
            w1_sb = [wp5.tile([128, DIL], BF16, tag=f"w1_{dc}", name=f"w1_{dc}") for dc in range(8)]
            w2_sb = [wp5.tile([128, D], BF16, tag=f"w2_{mc}", name=f"w2_{mc}") for mc in range(8)]
            for dc in range(8):
                nc.sync.dma_start(out=w1_sb[dc], in_=wff1T[dc * 128:(dc + 1) * 128, :])
                nc.sync.dma_start(out=w2_sb[dc], in_=wff2T[dc * 128:(dc + 1) * 128, :])
            for nb in range(2):
                hh_sb = [sp5.tile([128, 512], BF16, tag=f"hh{dc}", name=f"hh{dc}") for dc in range(8)]
                for dc in range(8):
                    half, dc4 = dc // 4, dc % 4
                    srcv = ag1_out[half][nb * 2:nb * 2 + 2, dc4 * 128:(dc4 + 1) * 128, :] \
                        .rearrange("b d t -> d b t")
                    nc.sync.dma_start(out=hh_sb[dc], in_=srcv)
                ffh = sp5.tile([128, 8, 512], BF16, tag="ffh")
                for mc in range(8):
                    ps = pp5.tile([128, 512], F32, tag="ps5a")
                    for dc in range(8):
                        nc.tensor.matmul(ps,
                                         lhsT=w1_sb[dc][:, mc * 128:(mc + 1) * 128],
                                         rhs=hh_sb[dc],
                                         start=(dc == 0), stop=(dc == 7))
                    nc.scalar.activation(out=ffh[:, mc, :], in_=ps, func=AF.Relu,
                                         bias=bff1_sb[:, mc:mc + 1], scale=1.0)
                for oc in range(8):
                    ps = pp5.tile([128, 512], F32, tag="ps5b")
                    for mc in range(8):
                        nc.tensor.matmul(ps,
                                         lhsT=w2_sb[mc][:, oc * 128:(oc + 1) * 128],
                                         rhs=ffh[:, mc, :],
                                         start=(mc == 0), stop=(mc == 7))
                    ot = sp5.tile([128, 512], BF16, tag="ot5")
                    nc.scalar.copy(ot, ps)
                    halfo, oc4 = oc // 4, oc % 4
                    dst = rs2_in[halfo][nb * 2:nb * 2 + 2, oc4 * 128:(oc4 + 1) * 128, :] \
                        .rearrange("b o t -> o b t")
                    nc.sync.dma_start(out=dst, in_=ot)
            for half in range(2):
                nc.gpsimd.collective_compute(
                    "ReduceScatter", mybir.AluOpType.add, replica_groups=RG,
                    ins=[rs2_in[half][:]], outs=[rs2_out[half][:]])

        # ---------------- Phase 6: final LN + output ----------------
        with tc.tile_pool(name="p6s", bufs=1) as sp6, \
             tc.tile_pool(name="p6ps", bufs=1, space="PSUM") as pp6:
            h2 = sp6.tile([128, 8, TL], F32, tag="h2")
            for dc in range(8):
                nc.vector.tensor_scalar_add(h2[:, dc, :], hsl[:, dc, :],
                                            bff2_sb[:, dc:dc + 1])
            xfb = sp6.tile([128, 8, TL], BF16, tag="xfb")
            for half in range(2):
                nc.sync.dma_start(
                    out=xfb[:, half * 4:half * 4 + 4, :],
                    in_=rs2_out[half].rearrange("(c p) t -> p c t", p=128))
            xf = sp6.tile([128, 8, TL], F32, tag="xf")
            nc.vector.tensor_copy(xf, xfb)
            nc.vector.tensor_add(xf, xf, h2)
            yt = sp6.tile([128, 8, TL], F32, tag="yt")
            layer_norm(sp6, pp6, xf, yt, None)
            nc.sync.dma_start(out=y_out.rearrange("(c p) t -> p c t", p=128),
                              in_=yt)

    nc.compile()
    return nc


def _stage(z, z_hist, u, pos_emb, W_qkv, W_r, r_w_bias, r_r_bias, W_o, b_o,
           W_ff1, b_ff1, W_ff2, b_ff2):
    f32 = np.float32
    bf16 = ml_dtypes.bfloat16
    cats = [np.ascontiguousarray(
        np.concatenate([z_hist[b], z[b]], axis=1)).astype(bf16) for b in range(B)]
    pos_w = np.ascontiguousarray(pos_emb[0, :, 1048:2048]).astype(bf16)
    ident_bf = np.eye(128).astype(bf16)
    ones_bv = np.ones((128, 1)).astype(bf16)
    onesr_v = np.ones((1, 128), dtype=f32)
    in_maps = []
    for c in range(N_CORES):
        b, g = c // 4, c % 4
        cs = 256 * g
        m = dict(
            cat=cats[b],
            wqT=np.ascontiguousarray(W_qkv[cs:cs + CO, :].T).astype(bf16),
            wkT=np.ascontiguousarray(W_qkv[1024 + cs:1024 + cs + CO, :].T).astype(bf16),
            wvT=np.ascontiguousarray(W_qkv[2048 + cs:2048 + cs + CO, :].T).astype(bf16),
            wrT=np.ascontiguousarray(W_r[cs:cs + CO, :].T).astype(bf16),
            u_q=np.ascontiguousarray(u[b, cs:cs + CO, M:]).astype(bf16),
            u_k=np.ascontiguousarray(u[b, 1024 + cs:1024 + cs + CO, :]).astype(bf16),
            u_vT=np.ascontiguousarray(u[b, 2048 + cs:2048 + cs + CO, :].T).astype(bf16),
            pos_w=pos_w,
            ident_b=ident_bf,
            ones_b=ones_bv,
            onesr=onesr_v,
            rwb=np.ascontiguousarray(
                r_w_bias[4 * g:4 * g + 4].reshape(CO).reshape(2, 128).T, dtype=f32),
            rrb=np.ascontiguousarray(
                r_r_bias[4 * g:4 * g + 4].reshape(CO).reshape(2, 128).T, dtype=f32),
            woT=np.ascontiguousarray(W_o[:, cs:cs + CO].T).astype(bf16),
            bo=np.ascontiguousarray(b_o.reshape(8, 128).T, dtype=f32),
            zres=np.ascontiguousarray(z[b, :, TL * g:TL * g + TL] + b_o[:, None], dtype=f32),
            wff1T=np.ascontiguousarray(W_ff1[DIL * g:DIL * g + DIL, :].T).astype(bf16),
            bff1=np.ascontiguousarray(
                b_ff1[DIL * g:DIL * g + DIL].reshape(8, 128).T, dtype=f32),
            wff2T=np.ascontiguousarray(W_ff2[:, DIL * g:DIL * g + DIL].T).astype(bf16),
            bff2=np.ascontiguousarray(b_ff2.reshape(8, 128).T, dtype=f32),
        )
        in_maps.append(m)
    return in_maps


def kernel(**inputs):
    if "nc" not in _CACHE:
        _CACHE["nc"] = _build()
    nc = _CACHE["nc"]
    in_maps = _stage(**inputs)
    res = bass_utils.run_bass_kernel_spmd(
        nc, in_maps, core_ids=list(range(N_CORES)))
    y = np.zeros((B, D, Q), dtype=np.float32)
    for c in range(N_CORES):
        b, g = c // 4, c % 4
        y[b, :, TL * g:TL * g + TL] = res.results[c]["y"]
    return y
